# revision 17
# baseline (speedup 1.0000x reference)
"""ChildSum TreeLSTM (complete binary tree, depth 17) on 8 Trainium2 NeuronCores.

Strategy (v2)
-------------
* 8 independent subtrees (roots 7..14), core m owns subtree 7+m, bottom-up
  levels 16..12 on device; levels 11..0 finished on the host in float64.
* Feature-major layout [128 hidden, nodes]; per level an "even/odd split"
  column order so every slice is contiguous (see _stored_cols).
* DEEP levels {16,15,14} run in fp8e4 with DoubleRow matmuls (0.5 cyc/col):
  each level-d buffer is [he | x | ho] fp8 so the gate GEMMs pair
  (U,W)@(he,x) and (W,U)@(x,ho) straight out of one tile.  i/o gates use
  exact sigmoid on the ACT engine (bias via K=1 f32r matmul), the f gates
  use a clamped-linear sigmoid on the DVE (slope 0.2231), u/tanh(c) exact
  on ACT.  Gates/c in bf16, h in fp8 (f32 for level 14's h, consumed by
  the f32r level-13 GEMMs).  Leaf i/o also use the clamped sigmoid.
* TOP levels {13,12} are plain f32r exactly like the v1 kernel.
* Elementwise work is split ACT/DVE/Pool to balance engine busy time; the
  tanh(c)+h of each chunk is deferred one chunk (software pipelining).
"""

import os
import sys

import numpy as np
import ml_dtypes

for _p in ("/opt/trn_rl_repo", "/root/.axon_site/_ro/trn_rl_repo"):
    if os.path.isdir(_p) and _p not in sys.path:
        sys.path.insert(0, _p)

import concourse.bacc as bacc
import concourse.tile as tile
from concourse import mybir
from concourse.bass_utils import run_bass_kernel_spmd

DEPTH = 17
N = 2**DEPTH - 1
H = 128
NCORES = 8
L_STOP = 12
CHUNK = 512
SLOPE = 0.2231  # clamped-linear sigmoid slope (minimax optimal)

DEV_LEVELS = list(range(DEPTH - 1, L_STOP - 1, -1))  # 16..12
DEEP = {16, 15, 14}
TOP = {13, 12}
LCOLS = {d: (2**d) // NCORES for d in DEV_LEVELS}
X8_LVLS = [16, 15, 14]
X32_LVLS = [13, 12]
X8OFF = {16: 0, 15: LCOLS[16], 14: LCOLS[16] + LCOLS[15]}
X8COLS = sum(LCOLS[d] for d in X8_LVLS)
X32OFF = {13: 0, 12: LCOLS[13]}
X32COLS = sum(LCOLS[d] for d in X32_LVLS)
TOPC = LCOLS[L_STOP]

F32 = mybir.dt.float32
F32R = mybir.dt.float32r
BF16 = mybir.dt.bfloat16
F8 = mybir.dt.float8e4
NP8 = ml_dtypes.float8_e4m3
NPBF = ml_dtypes.bfloat16

G8 = {"i": 0, "o": 384, "u": 768, "f": 1152}  # [U|W|U] blocks per gate
W_NAMES = ["Wi", "Wo", "Wu", "Wf", "Ui", "Uo", "Uu", "Uf"]
WOFF = {n: i * H for i, n in enumerate(W_NAMES)}

MUL = mybir.AluOpType.mult
ADD = mybir.AluOpType.add
AMAX = mybir.AluOpType.max
AMIN = mybir.AluOpType.min
DR = mybir.MatmulPerfMode.DoubleRow


def _build_nc():
    nc = bacc.Bacc("TRN2", target_bir_lowering=False, debug=False)
    xT8 = nc.dram_tensor("xT8", [H, X8COLS], F8, kind="ExternalInput").ap()
    xT32 = nc.dram_tensor("xT32", [H, X32COLS], F32R, kind="ExternalInput").ap()
    w8 = nc.dram_tensor("w8", [H, 12 * H], F8, kind="ExternalInput").ap()
    w32 = nc.dram_tensor("w32", [H, 8 * H], F32R, kind="ExternalInput").ap()
    bias = nc.dram_tensor("bias", [H, 4], F32, kind="ExternalInput").ap()
    bs = nc.dram_tensor("bs", [H, 4], F32, kind="ExternalInput").ap()
    # K=1 bias rows: [bWi+bUi | bWo+bUo | ones(CHUNK)]
    biasT = nc.dram_tensor("biasT", [1, 2 * H + CHUNK], F32R, kind="ExternalInput").ap()
    hc = nc.dram_tensor("hc", [H, 2 * TOPC], BF16, kind="ExternalOutput").ap()

    Sig = mybir.ActivationFunctionType.Sigmoid
    Tanh = mybir.ActivationFunctionType.Tanh

    with tile.TileContext(nc) as tc:
        with (
            tc.tile_pool(name="const", bufs=1) as constp,
            tc.tile_pool(name="xc", bufs=1) as xcp,
            tc.tile_pool(name="gates", bufs=3) as gp,
            tc.tile_pool(name="psio", bufs=1, space="PSUM") as psio,
            tc.tile_pool(name="psu", bufs=2, space="PSUM") as psu,
            tc.tile_pool(name="psf", bufs=2, space="PSUM") as psf,
        ):
            # first-needed tensors on the sync queue (SP has slack, SWDGE
            # descriptor generation costs Pool time); late ones on gpsimd
            w8_sb = constp.tile([H, 12 * H], F8, tag="w8")
            nc.sync.dma_start(out=w8_sb, in_=w8)
            b_sb = constp.tile([H, 4], F32, tag="b")
            nc.sync.dma_start(out=b_sb, in_=bias)
            bs_sb = constp.tile([H, 4], F32, tag="bs")
            nc.sync.dma_start(out=bs_sb, in_=bs)
            bT = constp.tile([1, 2 * H + CHUNK], F32R, tag="bT")
            nc.gpsimd.dma_start(out=bT, in_=biasT)
            ones = bT[:, 2 * H : 2 * H + CHUNK]
            w32_sb = constp.tile([H, 8 * H], F32R, tag="w32")
            nc.gpsimd.dma_start(out=w32_sb, in_=w32)
            warm = constp.tile([H, 1], F32, tag="warm")
            nc.vector.memset(warm, 0.0)
            nc.scalar.activation(warm, warm, Sig)

            # level buffers: deep levels hold [he | x | ho] fp8 (leaf: x only);
            # level 13/12 hold [he | x | ho] f32r.  c per level in bf16.
            xb = {}
            xb[16] = xcp.tile([H, LCOLS[16]], F8, tag="xb16", name="xb16")
            for d in (15, 14):
                xb[d] = xcp.tile([H, 3 * LCOLS[d]], F8, tag=f"xb{d}", name=f"xb{d}")
            for d in (13, 12):
                xb[d] = xcp.tile([H, 3 * LCOLS[d]], F32R, tag=f"xb{d}", name=f"xb{d}")
            cbuf = {
                d: xcp.tile([H, LCOLS[d]], BF16, tag=f"c{d}", name=f"c{d}")
                for d in DEV_LEVELS
            }
            hout = xcp.tile([H, TOPC], BF16, tag="hout", name="hout")

            # x loads: first two leaf segments ride the idle scalar/vector
            # queues in parallel with the const DMAs on sync; rest on sync
            seg = LCOLS[16] // 4
            nc.scalar.dma_start(out=xb[16][:, :seg], in_=xT8[:, :seg])
            nc.scalar.dma_start(
                out=xb[16][:, seg : 2 * seg], in_=xT8[:, seg : 2 * seg]
            )
            for d in (15, 14):
                L = LCOLS[d]
                nc.sync.dma_start(
                    out=xb[d][:, L : 2 * L],
                    in_=xT8[:, X8OFF[d] : X8OFF[d] + L],
                )
            for k in (2, 3):
                nc.sync.dma_start(
                    out=xb[16][:, k * seg : (k + 1) * seg],
                    in_=xT8[:, k * seg : (k + 1) * seg],
                )
            for d in (13, 12):
                L = LCOLS[d]
                nc.gpsimd.dma_start(
                    out=xb[d][:, L : 2 * L],
                    in_=xT32[:, X32OFF[d] : X32OFF[d] + L],
                )

            mm = nc.tensor.matmul
            act = nc.scalar.activation
            dve = nc.vector
            pool = nc.gpsimd

            def w8dr_uw(g):  # (U,W) stationary pair -> pairs rhs (he, x)
                return w8_sb[:, G8[g] : G8[g] + 2 * H].rearrange(
                    "p (two m) -> p two m", two=2
                )

            def w8dr_wu(g):  # (W,U) stationary pair -> pairs rhs (x, ho)
                return w8_sb[:, G8[g] + H : G8[g] + 3 * H].rearrange(
                    "p (two m) -> p two m", two=2
                )

            def w8u(g):  # plain U
                return w8_sb[:, G8[g] : G8[g] + H]

            def w8w(g):  # plain W
                return w8_sb[:, G8[g] + H : G8[g] + 2 * H]

            def w32sl(name):
                return w32_sb[:, WOFF[name] : WOFF[name] + H]

            # deferred tanh(c)+h of the previous chunk (software pipelining)
            pending = []

            def flush_pending():
                while pending:
                    dv, av, Cv, o_ap, h_dst, h_eng = pending.pop(0)
                    t_sb = gp.tile([H, CHUNK], BF16, tag="t_sb", name="t_sb")
                    if dv == DEPTH - 1:
                        # leaf: |c|<=1-ish, clamped-linear tanh on DVE
                        dve.tensor_scalar(
                            t_sb[:, :Cv], cbuf[dv][:, av : av + Cv],
                            -1.0, 1.0, AMAX, AMIN,
                        )
                    else:
                        act(t_sb[:, :Cv], cbuf[dv][:, av : av + Cv], Tanh)
                    h_eng.tensor_mul(h_dst, o_ap, t_sb[:, :Cv])

            def h_slot(d, a, C):
                """Destination slice for level-d h at (a, a+C) plus engine."""
                if d == L_STOP:
                    return hout[:, a : a + C], dve
                Lp = LCOLS[d - 1]
                if a < Lp:  # even child block of parent level
                    dst = xb[d - 1][:, a : a + C]
                else:  # odd child block
                    dst = xb[d - 1][:, 2 * Lp + (a - Lp) : 2 * Lp + (a - Lp) + C]
                return dst, (pool if d in DEEP else pool)

            for d in DEV_LEVELS:
                L = LCOLS[d]
                leaf = d == DEPTH - 1
                flush_pending()
                step = 256 if d == L_STOP else CHUNK
                for a in range(0, L, step):
                    C = min(step, L - a)
                    io_ps = psio.tile([H, 2 * CHUNK], F32, tag="io")
                    u_ps = psu.tile([H, CHUNK], F32, tag="u")
                    isl = io_ps[:, :C]
                    osl = io_ps[:, CHUNK : CHUNK + C]
                    if leaf:
                        x_sl = xb[16][:, a : a + C]
                        mm(isl, w8w("i"), x_sl, start=True, stop=True)
                        mm(osl, w8w("o"), x_sl, start=True, stop=True)
                        mm(u_ps[:, :C], w8w("u"), x_sl, start=True, stop=True)
                    elif d in DEEP:
                        x3 = xb[d].rearrange("p (three l) -> p three l", three=3)
                        hex_ = x3[:, 0:2, a : a + C]  # (he, x)
                        xho = x3[:, 1:3, a : a + C]  # (x, ho)
                        ho_f = xb[d][:, 2 * L + a : 2 * L + a + C]
                        for g, psl, bo in (("i", isl, 0), ("o", osl, H)):
                            mm(psl, w8dr_uw(g), hex_, start=True, stop=False,
                               perf_mode=DR)
                            mm(psl, w8u(g), ho_f, start=False, stop=False)
                            # exact-sigmoid io bias via K=1 f32r matmul
                            mm(psl, bT[:, bo : bo + H], ones[:, :C], start=False,
                               stop=True)
                        mm(u_ps[:, :C], w8dr_uw("u"), hex_, start=True, stop=False,
                           perf_mode=DR)
                        mm(u_ps[:, :C], w8u("u"), ho_f, start=False, stop=True)
                        f_ps = psf.tile([H, 2 * CHUNK], F32, tag="f")
                        mm(f_ps[:, :C], w8dr_uw("f"), hex_, start=True, stop=True,
                           perf_mode=DR)
                        mm(f_ps[:, CHUNK : CHUNK + C], w8dr_wu("f"), xho,
                           start=True, stop=True, perf_mode=DR)
                    else:  # top f32r levels
                        x_sl = xb[d][:, L + a : L + a + C]
                        he = xb[d][:, a : a + C]
                        ho = xb[d][:, 2 * L + a : 2 * L + a + C]
                        mm(isl, w32sl("Wi"), x_sl, start=True, stop=False)
                        mm(isl, w32sl("Ui"), he, start=False, stop=False)
                        mm(isl, w32sl("Ui"), ho, start=False, stop=False)
                        mm(isl, bT[:, 0:H], ones[:, :C], start=False, stop=True)
                        mm(osl, w32sl("Wo"), x_sl, start=True, stop=False)
                        mm(osl, w32sl("Uo"), he, start=False, stop=False)
                        mm(osl, w32sl("Uo"), ho, start=False, stop=False)
                        mm(osl, bT[:, H : 2 * H], ones[:, :C], start=False,
                           stop=True)
                        mm(u_ps[:, :C], w32sl("Wu"), x_sl, start=True, stop=False)
                        mm(u_ps[:, :C], w32sl("Uu"), he, start=False, stop=False)
                        mm(u_ps[:, :C], w32sl("Uu"), ho, start=False, stop=True)
                        f_ps = psf.tile([H, 2 * CHUNK], F32, tag="f")
                        f0 = f_ps[:, :C]
                        f1 = f_ps[:, CHUNK : CHUNK + C]
                        mm(f0, w32sl("Wf"), x_sl, start=True, stop=False)
                        mm(f1, w32sl("Wf"), x_sl, start=True, stop=False)
                        mm(f0, w32sl("Uf"), he, start=False, stop=True)
                        mm(f1, w32sl("Uf"), ho, start=False, stop=True)

                    io16 = gp.tile([H, 2 * CHUNK], BF16, tag="io16", name="io16")
                    u16 = gp.tile([H, CHUNK], BF16, tag="u16", name="u16")
                    i_sl = io16[:, :C]
                    o_sl = io16[:, CHUNK : CHUNK + C]
                    if leaf:
                        # i: clamped-linear sigmoid on DVE; o: exact on ACT
                        dve.tensor_scalar(i_sl, isl, SLOPE, bs_sb[:, 0:1], MUL, ADD)
                        dve.tensor_scalar(i_sl, i_sl, 0.0, 1.0, AMAX, AMIN)
                        act(o_sl, osl, Sig, bias=b_sb[:, 3:4])
                        act(u16[:, :C], u_ps[:, :C], Tanh, bias=b_sb[:, 0:1])
                        flush_pending()
                        c_sl = cbuf[d][:, a : a + C]
                        pool.tensor_mul(c_sl, i_sl, u16[:, :C])
                    else:
                        # exact io on ACT (bias already matmul'd into PSUM)
                        if C == CHUNK:
                            act(io16, io_ps, Sig)
                        else:
                            act(
                                io16.rearrange("p (two c) -> p two c", two=2)[:, :, :C],
                                io_ps.rearrange("p (two c) -> p two c", two=2)[:, :, :C],
                                Sig,
                            )
                        act(u16[:, :C], u_ps[:, :C], Tanh, bias=b_sb[:, 1:2])
                        f16 = gp.tile([H, 2 * CHUNK], BF16, tag="f16", name="f16")
                        if d in DEEP:
                            # hard sigmoid f on DVE: affine + clamp
                            if C == CHUNK:
                                dve.tensor_scalar(f16, f_ps, SLOPE, bs_sb[:, 2:3],
                                                  MUL, ADD)
                                dve.tensor_scalar(f16, f16, 0.0, 1.0, AMAX, AMIN)
                            else:
                                fv = f16.rearrange("p (two c) -> p two c", two=2)[:, :, :C]
                                pv = f_ps.rearrange("p (two c) -> p two c", two=2)[:, :, :C]
                                dve.tensor_scalar(fv, pv, SLOPE, bs_sb[:, 2:3],
                                                  MUL, ADD)
                                dve.tensor_scalar(fv, fv, 0.0, 1.0, AMAX, AMIN)
                        else:
                            if C == CHUNK:
                                act(f16, f_ps, Sig, bias=b_sb[:, 2:3])
                            else:
                                act(
                                    f16.rearrange("p (two c) -> p two c", two=2)[:, :, :C],
                                    f_ps.rearrange("p (two c) -> p two c", two=2)[:, :, :C],
                                    Sig,
                                    bias=b_sb[:, 2:3],
                                )
                        q = gp.tile([H, CHUNK], BF16, tag="q", name="q")
                        pr = gp.tile([H, 2 * CHUNK], BF16, tag="pr", name="pr")
                        s1 = gp.tile([H, CHUNK], BF16, tag="s1", name="s1")
                        pool.tensor_mul(q[:, :C], i_sl, u16[:, :C])
                        dve.tensor_mul(
                            pr.rearrange("p (two c) -> p two c", two=2)[:, :, :C],
                            f16.rearrange("p (two c) -> p two c", two=2)[:, :, :C],
                            cbuf[d + 1].rearrange("p (two l) -> p two l", two=2)[
                                :, :, a : a + C
                            ],
                        )
                        flush_pending()
                        pool.tensor_add(s1[:, :C], q[:, :C], pr[:, :C])
                        c_sl = cbuf[d][:, a : a + C]
                        pool.tensor_add(c_sl, s1[:, :C], pr[:, CHUNK : CHUNK + C])
                    h_dst, h_eng = h_slot(d, a, C)
                    pending.append((d, a, C, o_sl, h_dst, h_eng))

            nc.gpsimd.dma_start(out=hc[:, TOPC : 2 * TOPC], in_=cbuf[L_STOP])
            flush_pending()
            nc.sync.dma_start(out=hc[:, :TOPC], in_=hout)
    nc.finalize()
    return nc


_NC = None


def _get_nc():
    global _NC
    if _NC is None:
        _NC = _build_nc()
    return _NC


def _stored_cols(m):
    """Node ids per level for core m: per_level[d] in even/odd-split order."""
    ids = np.arange(2**L_STOP - 1 + TOPC * m, 2**L_STOP - 1 + TOPC * (m + 1))
    per_level = {L_STOP: ids}
    for d in range(L_STOP, DEPTH - 1):
        ids = np.concatenate([2 * ids + 1, 2 * ids + 2])
        per_level[d + 1] = ids
    return per_level


def _host_tensors(inputs):
    """Shared (per-core-identical) device tensors."""
    b = {k: np.asarray(inputs[k], np.float64) for k in inputs if k.startswith("b")}
    w8 = np.zeros((H, 12 * H), np.float32)
    for g, (wn, un) in (("i", ("Wi", "Ui")), ("o", ("Wo", "Uo")),
                        ("u", ("Wu", "Uu")), ("f", ("Wf", "Uf"))):
        U = np.asarray(inputs[un], np.float32).T
        W = np.asarray(inputs[wn], np.float32).T
        off = G8[g]
        w8[:, off : off + H] = U
        w8[:, off + H : off + 2 * H] = W
        w8[:, off + 2 * H : off + 3 * H] = U
    w8 = w8.astype(NP8)
    w32 = np.ascontiguousarray(
        np.concatenate([np.asarray(inputs[n], np.float32).T for n in W_NAMES], axis=1)
    )
    bias = np.zeros((H, 4), np.float32)
    bias[:, 0] = b["bWu"]
    bias[:, 1] = b["bWu"] + b["bUu"]
    bias[:, 2] = b["bWf"] + b["bUf"]
    bias[:, 3] = b["bWo"]
    bs = np.zeros((H, 4), np.float32)
    bs[:, 0] = SLOPE * b["bWi"] + 0.5
    bs[:, 1] = SLOPE * b["bWo"] + 0.5
    bs[:, 2] = SLOPE * (b["bWf"] + b["bUf"]) + 0.5
    biasT = np.zeros((1, 2 * H + CHUNK), np.float32)
    biasT[0, 0:H] = b["bWi"] + b["bUi"]
    biasT[0, H : 2 * H] = b["bWo"] + b["bUo"]
    biasT[0, 2 * H :] = 1.0
    return w8, w32, bias, bs, biasT


def _core_inputs(x, m, shared):
    w8, w32, bias, bs, biasT = shared
    per_level = _stored_cols(m)
    x8 = np.concatenate([x[per_level[d]] for d in X8_LVLS]).T
    x32 = np.concatenate([x[per_level[d]] for d in X32_LVLS]).T
    return {
        "xT8": np.ascontiguousarray(x8.astype(NP8)),
        "xT32": np.ascontiguousarray(x32.astype(np.float32)),
        "w8": w8,
        "w32": w32,
        "bias": bias,
        "bs": bs,
        "biasT": biasT,
    }


def _sigmoid(z):
    return 1.0 / (1.0 + np.exp(-z))


def kernel(**inputs):
    x = np.ascontiguousarray(np.asarray(inputs["x"], dtype=np.float32))
    shared = _host_tensors(inputs)
    in_maps = [_core_inputs(x, m, shared) for m in range(NCORES)]

    nc = _get_nc()
    trace = bool(int(os.environ.get("KERNEL_TRACE", "0")))
    try:
        res = run_bass_kernel_spmd(
            nc, in_maps, core_ids=list(range(NCORES)), trace=trace
        )
    except ModuleNotFoundError:
        res = run_bass_kernel_spmd(nc, in_maps, core_ids=list(range(NCORES)))
    if trace and res.exec_time_ns is not None:
        print(f"HW exec time: {res.exec_time_ns} ns")

    h_next = np.concatenate(
        [np.asarray(res.results[m]["hc"][:, :TOPC], np.float64) for m in range(NCORES)],
        axis=1,
    ).T
    c_next = np.concatenate(
        [
            np.asarray(res.results[m]["hc"][:, TOPC : 2 * TOPC], np.float64)
            for m in range(NCORES)
        ],
        axis=1,
    ).T

    b = {k: np.asarray(inputs[k], np.float64) for k in inputs if k.startswith("b")}
    xd = x.astype(np.float64)
    W = {n: np.asarray(inputs[n], np.float64) for n in W_NAMES}
    for d in range(L_STOP - 1, -1, -1):
        s = 2**d
        cnt = 2**d
        s = s - 1
        xs = xd[s : s + cnt]
        li = xs @ W["Wi"].T + b["bWi"]
        lf = xs @ W["Wf"].T + b["bWf"]
        lo = xs @ W["Wo"].T + b["bWo"]
        lu = xs @ W["Wu"].T + b["bWu"]
        ch_h = h_next.reshape(cnt, 2, H)
        ch_c = c_next.reshape(cnt, 2, H)
        hs = ch_h[:, 0, :] + ch_h[:, 1, :]
        i = _sigmoid(li + hs @ W["Ui"].T + b["bUi"])
        o = _sigmoid(lo + hs @ W["Uo"].T + b["bUo"])
        u = np.tanh(lu + hs @ W["Uu"].T + b["bUu"])
        f0 = _sigmoid(lf + ch_h[:, 0, :] @ W["Uf"].T + b["bUf"])
        f1 = _sigmoid(lf + ch_h[:, 1, :] @ W["Uf"].T + b["bUf"])
        c = i * u + f0 * ch_c[:, 0, :] + f1 * ch_c[:, 1, :]
        h = o * np.tanh(c)
        h_next, c_next = h, c

    out = h_next[0] @ np.asarray(inputs["Wp"], np.float64).T + np.asarray(
        inputs["bWp"], np.float64
    )
    return out.astype(np.float32)


# revision 18
# speedup vs baseline: 1.2205x; 1.2205x over previous
"""ChildSum TreeLSTM (complete binary tree, depth 17) on 8 Trainium2 NeuronCores.

Strategy (v2)
-------------
* 8 independent subtrees (roots 7..14), core m owns subtree 7+m, bottom-up
  levels 16..12 on device; levels 11..0 finished on the host in float64.
* Feature-major layout [128 hidden, nodes]; per level an "even/odd split"
  column order so every slice is contiguous (see _stored_cols).
* DEEP levels {16,15,14} run in fp8e4 with DoubleRow matmuls (0.5 cyc/col):
  each level-d buffer is [he | x | ho] fp8 so the gate GEMMs pair
  (U,W)@(he,x) and (W,U)@(x,ho) straight out of one tile.  i/o gates use
  exact sigmoid on the ACT engine (bias via K=1 f32r matmul), the f gates
  use a clamped-linear sigmoid on the DVE (slope 0.2231), u/tanh(c) exact
  on ACT.  Gates/c in bf16, h in fp8 (f32 for level 14's h, consumed by
  the f32r level-13 GEMMs).  Leaf i/o also use the clamped sigmoid.
* TOP levels {13,12} are plain f32r exactly like the v1 kernel.
* Elementwise work is split ACT/DVE/Pool to balance engine busy time; the
  tanh(c)+h of each chunk is deferred one chunk (software pipelining).
"""

import os
import sys

import numpy as np
import ml_dtypes

for _p in ("/opt/trn_rl_repo", "/root/.axon_site/_ro/trn_rl_repo"):
    if os.path.isdir(_p) and _p not in sys.path:
        sys.path.insert(0, _p)

import concourse.bacc as bacc
import concourse.tile as tile
from concourse import mybir
from concourse.bass_utils import run_bass_kernel_spmd

DEPTH = 17
N = 2**DEPTH - 1
H = 128
NCORES = 8
L_STOP = 12
CHUNK = 512
SLOPE = 0.2231  # clamped-linear sigmoid slope (minimax optimal)

DEV_LEVELS = list(range(DEPTH - 1, L_STOP - 1, -1))  # 16..12
DEEP = {16, 15, 14}
TOP = {13, 12}
LCOLS = {d: (2**d) // NCORES for d in DEV_LEVELS}
X8_LVLS = [16, 15, 14]
X32_LVLS = [13, 12]
X8OFF = {16: 0, 15: LCOLS[16], 14: LCOLS[16] + LCOLS[15]}
X8COLS = sum(LCOLS[d] for d in X8_LVLS)
X32OFF = {13: 0, 12: LCOLS[13]}
X32COLS = sum(LCOLS[d] for d in X32_LVLS)
TOPC = LCOLS[L_STOP]

F32 = mybir.dt.float32
F32R = mybir.dt.float32r
BF16 = mybir.dt.bfloat16
F8 = mybir.dt.float8e4
NP8 = ml_dtypes.float8_e4m3
NPBF = ml_dtypes.bfloat16

G8 = {"i": 0, "o": 384, "u": 768, "f": 1152}  # [U|W|U] blocks per gate
W_NAMES = ["Wi", "Wo", "Wu", "Wf", "Ui", "Uo", "Uu", "Uf"]
WOFF = {n: i * H for i, n in enumerate(W_NAMES)}

MUL = mybir.AluOpType.mult
ADD = mybir.AluOpType.add
AMAX = mybir.AluOpType.max
AMIN = mybir.AluOpType.min
DR = mybir.MatmulPerfMode.DoubleRow


def _build_nc():
    nc = bacc.Bacc("TRN2", target_bir_lowering=False, debug=False)
    xT8 = nc.dram_tensor("xT8", [H, X8COLS], F8, kind="ExternalInput").ap()
    xT32 = nc.dram_tensor("xT32", [H, X32COLS], F32R, kind="ExternalInput").ap()
    w8 = nc.dram_tensor("w8", [H, 12 * H], F8, kind="ExternalInput").ap()
    w32 = nc.dram_tensor("w32", [H, 8 * H], F32R, kind="ExternalInput").ap()
    bias = nc.dram_tensor("bias", [H, 4], F32, kind="ExternalInput").ap()
    bs = nc.dram_tensor("bs", [H, 4], F32, kind="ExternalInput").ap()
    # K=1 bias rows: [bWi+bUi | bWo+bUo | ones(CHUNK)]
    biasT = nc.dram_tensor("biasT", [1, 2 * H + CHUNK], F32R, kind="ExternalInput").ap()
    hc = nc.dram_tensor("hc", [H, 2 * TOPC], BF16, kind="ExternalOutput").ap()

    Sig = mybir.ActivationFunctionType.Sigmoid
    Tanh = mybir.ActivationFunctionType.Tanh

    with tile.TileContext(nc) as tc:
        with (
            tc.tile_pool(name="const", bufs=1) as constp,
            tc.tile_pool(name="xc", bufs=1) as xcp,
            tc.tile_pool(name="gates", bufs=3) as gp,
            tc.tile_pool(name="psio", bufs=2, space="PSUM") as psio,
            tc.tile_pool(name="psu", bufs=2, space="PSUM") as psu,
            tc.tile_pool(name="psf", bufs=1, space="PSUM") as psf,
        ):
            # first-needed tensors on the sync queue (SP has slack, SWDGE
            # descriptor generation costs Pool time); late ones on gpsimd
            w8_sb = constp.tile([H, 12 * H], F8, tag="w8")
            nc.sync.dma_start(out=w8_sb, in_=w8)
            b_sb = constp.tile([H, 4], F32, tag="b")
            nc.sync.dma_start(out=b_sb, in_=bias)
            bs_sb = constp.tile([H, 4], F32, tag="bs")
            nc.sync.dma_start(out=bs_sb, in_=bs)
            bT = constp.tile([1, 2 * H + CHUNK], F32R, tag="bT")
            nc.gpsimd.dma_start(out=bT, in_=biasT)
            ones = bT[:, 2 * H : 2 * H + CHUNK]
            w32_sb = constp.tile([H, 8 * H], F32R, tag="w32")
            nc.gpsimd.dma_start(out=w32_sb, in_=w32)
            warm = constp.tile([H, 1], F32, tag="warm")
            nc.vector.memset(warm, 0.0)
            nc.scalar.activation(warm, warm, Sig)

            # level buffers: deep levels hold [he | x | ho] fp8 (leaf: x only);
            # level 13/12 hold [he | x | ho] f32r.  c per level in bf16.
            xb = {}
            xb[16] = xcp.tile([H, LCOLS[16]], F8, tag="xb16", name="xb16")
            for d in (15, 14):
                xb[d] = xcp.tile([H, 3 * LCOLS[d]], F8, tag=f"xb{d}", name=f"xb{d}")
            for d in (13, 12):
                xb[d] = xcp.tile([H, 3 * LCOLS[d]], F32R, tag=f"xb{d}", name=f"xb{d}")
            cbuf = {
                d: xcp.tile([H, LCOLS[d]], BF16, tag=f"c{d}", name=f"c{d}")
                for d in DEV_LEVELS
            }
            hout = xcp.tile([H, TOPC], BF16, tag="hout", name="hout")

            # x loads: first two leaf segments ride the idle scalar/vector
            # queues in parallel with the const DMAs on sync; rest on sync
            seg = LCOLS[16] // 4
            nc.scalar.dma_start(out=xb[16][:, :seg], in_=xT8[:, :seg])
            nc.scalar.dma_start(
                out=xb[16][:, seg : 2 * seg], in_=xT8[:, seg : 2 * seg]
            )
            for d in (15, 14):
                L = LCOLS[d]
                nc.sync.dma_start(
                    out=xb[d][:, L : 2 * L],
                    in_=xT8[:, X8OFF[d] : X8OFF[d] + L],
                )
            for k in (2, 3):
                nc.sync.dma_start(
                    out=xb[16][:, k * seg : (k + 1) * seg],
                    in_=xT8[:, k * seg : (k + 1) * seg],
                )
            for d in (13, 12):
                L = LCOLS[d]
                nc.gpsimd.dma_start(
                    out=xb[d][:, L : 2 * L],
                    in_=xT32[:, X32OFF[d] : X32OFF[d] + L],
                )

            mm = nc.tensor.matmul
            act = nc.scalar.activation
            dve = nc.vector
            pool = nc.gpsimd

            def w8dr_uw(g):  # (U,W) stationary pair -> pairs rhs (he, x)
                return w8_sb[:, G8[g] : G8[g] + 2 * H].rearrange(
                    "p (two m) -> p two m", two=2
                )

            def w8dr_wu(g):  # (W,U) stationary pair -> pairs rhs (x, ho)
                return w8_sb[:, G8[g] + H : G8[g] + 3 * H].rearrange(
                    "p (two m) -> p two m", two=2
                )

            def w8u(g):  # plain U
                return w8_sb[:, G8[g] : G8[g] + H]

            def w8w(g):  # plain W
                return w8_sb[:, G8[g] + H : G8[g] + 2 * H]

            def w32sl(name):
                return w32_sb[:, WOFF[name] : WOFF[name] + H]

            # deferred tanh(c)+h of the previous chunk (software pipelining)
            pending = []

            def flush_pending():
                while pending:
                    dv, av, Cv, o_ap, h_dst, h_eng = pending.pop(0)
                    t_sb = gp.tile([H, CHUNK], BF16, tag="t_sb", name="t_sb")
                    if dv == DEPTH - 1:
                        # leaf: |c|<=1-ish, clamped-linear tanh on DVE
                        dve.tensor_scalar(
                            t_sb[:, :Cv], cbuf[dv][:, av : av + Cv],
                            -1.0, 1.0, AMAX, AMIN,
                        )
                    else:
                        act(t_sb[:, :Cv], cbuf[dv][:, av : av + Cv], Tanh)
                    h_eng.tensor_mul(h_dst, o_ap, t_sb[:, :Cv])

            def h_slot(d, a, C):
                """Destination slice for level-d h at (a, a+C) plus engine."""
                if d == L_STOP:
                    return hout[:, a : a + C], dve
                Lp = LCOLS[d - 1]
                if a < Lp:  # even child block of parent level
                    dst = xb[d - 1][:, a : a + C]
                else:  # odd child block
                    dst = xb[d - 1][:, 2 * Lp + (a - Lp) : 2 * Lp + (a - Lp) + C]
                return dst, (pool if d in DEEP else pool)

            for d in DEV_LEVELS:
                L = LCOLS[d]
                leaf = d == DEPTH - 1
                flush_pending()
                step = 256 if d == L_STOP else CHUNK
                for a in range(0, L, step):
                    C = min(step, L - a)
                    io_ps = psio.tile([H, 2 * CHUNK], F32, tag="io")
                    u_ps = psu.tile([H, CHUNK], F32, tag="u")
                    isl = io_ps[:, :C]
                    osl = io_ps[:, CHUNK : CHUNK + C]
                    if leaf:
                        x_sl = xb[16][:, a : a + C]
                        mm(isl, w8w("i"), x_sl, start=True, stop=True)
                        mm(osl, w8w("o"), x_sl, start=True, stop=True)
                        mm(u_ps[:, :C], w8w("u"), x_sl, start=True, stop=True)
                    elif d in DEEP:
                        x3 = xb[d].rearrange("p (three l) -> p three l", three=3)
                        hex_ = x3[:, 0:2, a : a + C]  # (he, x)
                        xho = x3[:, 1:3, a : a + C]  # (x, ho)
                        ho_f = xb[d][:, 2 * L + a : 2 * L + a + C]
                        for g, psl, bo in (("i", isl, 0), ("o", osl, H)):
                            mm(psl, w8dr_uw(g), hex_, start=True, stop=False,
                               perf_mode=DR)
                            mm(psl, w8u(g), ho_f, start=False, stop=False)
                            # exact-sigmoid io bias via K=1 f32r matmul
                            mm(psl, bT[:, bo : bo + H], ones[:, :C], start=False,
                               stop=True)
                        mm(u_ps[:, :C], w8dr_uw("u"), hex_, start=True, stop=False,
                           perf_mode=DR)
                        mm(u_ps[:, :C], w8u("u"), ho_f, start=False, stop=True)
                        f_ps = psf.tile([H, 2 * CHUNK], F32, tag="f")
                        mm(f_ps[:, :C], w8dr_uw("f"), hex_, start=True, stop=True,
                           perf_mode=DR)
                        mm(f_ps[:, CHUNK : CHUNK + C], w8dr_wu("f"), xho,
                           start=True, stop=True, perf_mode=DR)
                    else:  # top f32r levels
                        x_sl = xb[d][:, L + a : L + a + C]
                        he = xb[d][:, a : a + C]
                        ho = xb[d][:, 2 * L + a : 2 * L + a + C]
                        mm(isl, w32sl("Wi"), x_sl, start=True, stop=False)
                        mm(isl, w32sl("Ui"), he, start=False, stop=False)
                        mm(isl, w32sl("Ui"), ho, start=False, stop=False)
                        mm(isl, bT[:, 0:H], ones[:, :C], start=False, stop=True)
                        mm(osl, w32sl("Wo"), x_sl, start=True, stop=False)
                        mm(osl, w32sl("Uo"), he, start=False, stop=False)
                        mm(osl, w32sl("Uo"), ho, start=False, stop=False)
                        mm(osl, bT[:, H : 2 * H], ones[:, :C], start=False,
                           stop=True)
                        mm(u_ps[:, :C], w32sl("Wu"), x_sl, start=True, stop=False)
                        mm(u_ps[:, :C], w32sl("Uu"), he, start=False, stop=False)
                        mm(u_ps[:, :C], w32sl("Uu"), ho, start=False, stop=True)
                        f_ps = psf.tile([H, 2 * CHUNK], F32, tag="f")
                        f0 = f_ps[:, :C]
                        f1 = f_ps[:, CHUNK : CHUNK + C]
                        mm(f0, w32sl("Wf"), x_sl, start=True, stop=False)
                        mm(f1, w32sl("Wf"), x_sl, start=True, stop=False)
                        mm(f0, w32sl("Uf"), he, start=False, stop=True)
                        mm(f1, w32sl("Uf"), ho, start=False, stop=True)

                    io16 = gp.tile([H, 2 * CHUNK], BF16, tag="io16", name="io16")
                    u16 = gp.tile([H, CHUNK], BF16, tag="u16", name="u16")
                    i_sl = io16[:, :C]
                    o_sl = io16[:, CHUNK : CHUNK + C]
                    if leaf:
                        # i: clamped-linear sigmoid on DVE; o: exact on ACT
                        dve.tensor_scalar(i_sl, isl, SLOPE, bs_sb[:, 0:1], MUL, ADD)
                        dve.tensor_scalar(i_sl, i_sl, 0.0, 1.0, AMAX, AMIN)
                        act(o_sl, osl, Sig, bias=b_sb[:, 3:4])
                        act(u16[:, :C], u_ps[:, :C], Tanh, bias=b_sb[:, 0:1])
                        flush_pending()
                        c_sl = cbuf[d][:, a : a + C]
                        pool.tensor_mul(c_sl, i_sl, u16[:, :C])
                    else:
                        # exact io on ACT (bias already matmul'd into PSUM)
                        if C == CHUNK:
                            act(io16, io_ps, Sig)
                        else:
                            act(
                                io16.rearrange("p (two c) -> p two c", two=2)[:, :, :C],
                                io_ps.rearrange("p (two c) -> p two c", two=2)[:, :, :C],
                                Sig,
                            )
                        act(u16[:, :C], u_ps[:, :C], Tanh, bias=b_sb[:, 1:2])
                        f16 = gp.tile([H, 2 * CHUNK], BF16, tag="f16", name="f16")
                        if d in DEEP:
                            # hard sigmoid f on DVE: affine + clamp
                            if C == CHUNK:
                                dve.tensor_scalar(f16, f_ps, SLOPE, bs_sb[:, 2:3],
                                                  MUL, ADD)
                                dve.tensor_scalar(f16, f16, 0.0, 1.0, AMAX, AMIN)
                            else:
                                fv = f16.rearrange("p (two c) -> p two c", two=2)[:, :, :C]
                                pv = f_ps.rearrange("p (two c) -> p two c", two=2)[:, :, :C]
                                dve.tensor_scalar(fv, pv, SLOPE, bs_sb[:, 2:3],
                                                  MUL, ADD)
                                dve.tensor_scalar(fv, fv, 0.0, 1.0, AMAX, AMIN)
                        else:
                            if C == CHUNK:
                                act(f16, f_ps, Sig, bias=b_sb[:, 2:3])
                            else:
                                act(
                                    f16.rearrange("p (two c) -> p two c", two=2)[:, :, :C],
                                    f_ps.rearrange("p (two c) -> p two c", two=2)[:, :, :C],
                                    Sig,
                                    bias=b_sb[:, 2:3],
                                )
                        q = gp.tile([H, CHUNK], BF16, tag="q", name="q")
                        pr = gp.tile([H, 2 * CHUNK], BF16, tag="pr", name="pr")
                        s1 = gp.tile([H, CHUNK], BF16, tag="s1", name="s1")
                        pool.tensor_mul(q[:, :C], i_sl, u16[:, :C])
                        dve.tensor_mul(
                            pr.rearrange("p (two c) -> p two c", two=2)[:, :, :C],
                            f16.rearrange("p (two c) -> p two c", two=2)[:, :, :C],
                            cbuf[d + 1].rearrange("p (two l) -> p two l", two=2)[
                                :, :, a : a + C
                            ],
                        )
                        flush_pending()
                        pool.tensor_add(s1[:, :C], q[:, :C], pr[:, :C])
                        c_sl = cbuf[d][:, a : a + C]
                        pool.tensor_add(c_sl, s1[:, :C], pr[:, CHUNK : CHUNK + C])
                    h_dst, h_eng = h_slot(d, a, C)
                    pending.append((d, a, C, o_sl, h_dst, h_eng))

            nc.gpsimd.dma_start(out=hc[:, TOPC : 2 * TOPC], in_=cbuf[L_STOP])
            flush_pending()
            nc.sync.dma_start(out=hc[:, :TOPC], in_=hout)
    nc.finalize()
    return nc


_NC = None


def _get_nc():
    global _NC
    if _NC is None:
        _NC = _build_nc()
    return _NC


def _stored_cols(m):
    """Node ids per level for core m: per_level[d] in even/odd-split order."""
    ids = np.arange(2**L_STOP - 1 + TOPC * m, 2**L_STOP - 1 + TOPC * (m + 1))
    per_level = {L_STOP: ids}
    for d in range(L_STOP, DEPTH - 1):
        ids = np.concatenate([2 * ids + 1, 2 * ids + 2])
        per_level[d + 1] = ids
    return per_level


def _host_tensors(inputs):
    """Shared (per-core-identical) device tensors."""
    b = {k: np.asarray(inputs[k], np.float64) for k in inputs if k.startswith("b")}
    w8 = np.zeros((H, 12 * H), np.float32)
    for g, (wn, un) in (("i", ("Wi", "Ui")), ("o", ("Wo", "Uo")),
                        ("u", ("Wu", "Uu")), ("f", ("Wf", "Uf"))):
        U = np.asarray(inputs[un], np.float32).T
        W = np.asarray(inputs[wn], np.float32).T
        off = G8[g]
        w8[:, off : off + H] = U
        w8[:, off + H : off + 2 * H] = W
        w8[:, off + 2 * H : off + 3 * H] = U
    w8 = w8.astype(NP8)
    w32 = np.ascontiguousarray(
        np.concatenate([np.asarray(inputs[n], np.float32).T for n in W_NAMES], axis=1)
    )
    bias = np.zeros((H, 4), np.float32)
    bias[:, 0] = b["bWu"]
    bias[:, 1] = b["bWu"] + b["bUu"]
    bias[:, 2] = b["bWf"] + b["bUf"]
    bias[:, 3] = b["bWo"]
    bs = np.zeros((H, 4), np.float32)
    bs[:, 0] = SLOPE * b["bWi"] + 0.5
    bs[:, 1] = SLOPE * b["bWo"] + 0.5
    bs[:, 2] = SLOPE * (b["bWf"] + b["bUf"]) + 0.5
    biasT = np.zeros((1, 2 * H + CHUNK), np.float32)
    biasT[0, 0:H] = b["bWi"] + b["bUi"]
    biasT[0, H : 2 * H] = b["bWo"] + b["bUo"]
    biasT[0, 2 * H :] = 1.0
    return w8, w32, bias, bs, biasT


def _core_inputs(x, m, shared):
    w8, w32, bias, bs, biasT = shared
    per_level = _stored_cols(m)
    x8 = np.concatenate([x[per_level[d]] for d in X8_LVLS]).T
    x32 = np.concatenate([x[per_level[d]] for d in X32_LVLS]).T
    return {
        "xT8": np.ascontiguousarray(x8.astype(NP8)),
        "xT32": np.ascontiguousarray(x32.astype(np.float32)),
        "w8": w8,
        "w32": w32,
        "bias": bias,
        "bs": bs,
        "biasT": biasT,
    }


def _sigmoid(z):
    return 1.0 / (1.0 + np.exp(-z))


def kernel(**inputs):
    x = np.ascontiguousarray(np.asarray(inputs["x"], dtype=np.float32))
    shared = _host_tensors(inputs)
    in_maps = [_core_inputs(x, m, shared) for m in range(NCORES)]

    nc = _get_nc()
    trace = bool(int(os.environ.get("KERNEL_TRACE", "0")))
    try:
        res = run_bass_kernel_spmd(
            nc, in_maps, core_ids=list(range(NCORES)), trace=trace
        )
    except ModuleNotFoundError:
        res = run_bass_kernel_spmd(nc, in_maps, core_ids=list(range(NCORES)))
    if trace and res.exec_time_ns is not None:
        print(f"HW exec time: {res.exec_time_ns} ns")

    h_next = np.concatenate(
        [np.asarray(res.results[m]["hc"][:, :TOPC], np.float64) for m in range(NCORES)],
        axis=1,
    ).T
    c_next = np.concatenate(
        [
            np.asarray(res.results[m]["hc"][:, TOPC : 2 * TOPC], np.float64)
            for m in range(NCORES)
        ],
        axis=1,
    ).T

    b = {k: np.asarray(inputs[k], np.float64) for k in inputs if k.startswith("b")}
    xd = x.astype(np.float64)
    W = {n: np.asarray(inputs[n], np.float64) for n in W_NAMES}
    for d in range(L_STOP - 1, -1, -1):
        s = 2**d
        cnt = 2**d
        s = s - 1
        xs = xd[s : s + cnt]
        li = xs @ W["Wi"].T + b["bWi"]
        lf = xs @ W["Wf"].T + b["bWf"]
        lo = xs @ W["Wo"].T + b["bWo"]
        lu = xs @ W["Wu"].T + b["bWu"]
        ch_h = h_next.reshape(cnt, 2, H)
        ch_c = c_next.reshape(cnt, 2, H)
        hs = ch_h[:, 0, :] + ch_h[:, 1, :]
        i = _sigmoid(li + hs @ W["Ui"].T + b["bUi"])
        o = _sigmoid(lo + hs @ W["Uo"].T + b["bUo"])
        u = np.tanh(lu + hs @ W["Uu"].T + b["bUu"])
        f0 = _sigmoid(lf + ch_h[:, 0, :] @ W["Uf"].T + b["bUf"])
        f1 = _sigmoid(lf + ch_h[:, 1, :] @ W["Uf"].T + b["bUf"])
        c = i * u + f0 * ch_c[:, 0, :] + f1 * ch_c[:, 1, :]
        h = o * np.tanh(c)
        h_next, c_next = h, c

    out = h_next[0] @ np.asarray(inputs["Wp"], np.float64).T + np.asarray(
        inputs["bWp"], np.float64
    )
    return out.astype(np.float32)
